# revision 36
# baseline (speedup 1.0000x reference)
"""Cross-attention kernel for Trainium2, 8 NeuronCores, data-parallel over batch.

Reference math per batch b:
    q_proj = q[b] @ Wq;  k_proj = y[b] @ Wk;  v_proj = k_proj @ Wv
    out = softmax(q_proj @ k_proj.T / 32) @ v_proj

Host-side restructure (kills the k-projection entirely, -14% device FLOPs):
    Wqk = Wq @ Wk.T   ->  scores = (q @ Wqk) @ y.T / 32
    Wkv = Wk @ Wv     ->  v_proj = y @ Wkv

Device per core (one batch per core, feature-major "T" layouts, no
on-device transposes):
    g   = q @ (16*Wqk)            bf16 matmul          [d', n] in PSUM (=16g)
    g8  = fp8e4(psum[0:KD])       ScalarE quantize     (fp8 half)
    gbf = bf16(16*psum[KD:])      ScalarE scale-copy   (bf16 half, =256g)
    v   = y @ Wkv                 bf16 matmul          [m, f] bf16
    S   = y8[:KD].T @ g8  (fp8 DoubleRow)  +  y[KD:].T @ gbf  (bf16)
    eT  = exp(S / 8192)           ScalarE              [m, n] bf16
    out = (eT.T @ v) / (eT.T @ 1) bf16 + ones-matmul denominator

fp8e4 DoubleRow contracts 256/instr at the same per-column rate as bf16
-> 2x throughput per pass (measured on hw).  A full-d single fp8 pass
costs 1.9e-2 rel err (too close to the 2e-2 gate); quantizing KD=768 of
the 1024-deep contraction and doing the rest in bf16 lands at 1.55e-2
with 5/8 of the bf16 score cost.  y is quantized on host (scaled by 16
to dodge the e4m3 subnormal range; the scale folds into exp's 1/8192).

All inputs are shipped in exact SBUF layout ([128, bytes] per-partition
blits, Wqk e-chunk-major) and striped across the sync+gpsimd DMA queues
in consumption order, so the tensor engine starts ~7us into the NEFF and
stays >92% busy.  Warmup matmuls ramp the PE clock during the first DMA.
"""

import numpy as np
import ml_dtypes
from contextlib import ExitStack

import concourse.bass as bass
import concourse.tile as tile
from concourse import bacc, mybir
from concourse.bass_utils import run_bass_kernel_spmd

P = 128
F32 = mybir.dt.float32
BF16 = mybir.dt.bfloat16
FP8 = mybir.dt.float8e4
E4NP = ml_dtypes.float8_e4m3
BF16NP = ml_dtypes.bfloat16

# Problem shapes (hardcoded per contract)
B = 8
NQ = 2048
NK = 2048
D = 1024   # in_q_dim == in_dim == hid_q == out_dim
F = 1024

YSCALE = 16.0   # host folds into y8;   |16*y|  < ~90  (e4m3 max 240)
GSCALE = 16.0   # host folds into Wqk;  |16*g|  < ~40
EXP_SCALE = 1.0 / (YSCALE * GSCALE * 32.0)  # exp((S_psum)/8192)
KD = 768        # d-range [0:KD) of the scores contraction runs fp8-DR


def build_program(nq=NQ, nk=NK, d=D, f=F, nblk=512):
    nc = bacc.Bacc(trn_type="TRN2")

    DC = d // P            # contraction chunks (8)
    KC = KD // P           # fp8 chunks of the scores contraction (4)
    MC = nk // P           # key chunks (16)
    NB = nq // nblk        # query blocks (4)
    NSUB = nblk // P       # 128-row subblocks per query block (4)
    FB = f // 512          # value free blocks (2)

    # Inputs are pre-arranged on host into exact SBUF layout [128, ...] so
    # every input DMA is a contiguous per-partition blit (multi-KB lines).
    NB_ = nq // nblk
    qT = nc.dram_tensor("qT", [NB_, P, DC * nblk], BF16, kind="ExternalInput").ap()
    yT = nc.dram_tensor("yT", [P, DC * nk], BF16, kind="ExternalInput").ap()
    y8T = nc.dram_tensor("y8T", [P, KC * nk], FP8, kind="ExternalInput").ap()
    Wqk = nc.dram_tensor("Wqk", [P, DC * d], BF16, kind="ExternalInput").ap()
    Wkv = nc.dram_tensor("Wkv", [P, DC * f], BF16, kind="ExternalInput").ap()
    out = nc.dram_tensor("out", [nq, f], F32, kind="ExternalOutput").ap()

    qT_v = qT.rearrange("b p (c n) -> b p c n", c=DC)
    yT_v = yT.rearrange("p (c m) -> p c m", c=DC)
    y8_v = y8T.rearrange("p (c m) -> p c m", c=KC)
    # Wqk host layout is d-chunk-major [p, di, ei, el] so block 0's g-phase
    # can stream d-major: pass di needs only the di-th 256KB piece.
    Wqk_v = Wqk.rearrange("p (c e l) -> p c e l", c=DC, e=DC)
    Wkv_v = Wkv.rearrange("p (c f) -> p c f", c=DC)
    out_v = out.rearrange("(b p) f -> b p f", p=P)

    with tile.TileContext(nc) as tc, ExitStack() as ctx:
        consts = ctx.enter_context(tc.tile_pool(name="consts", bufs=1))
        y8_pool = ctx.enter_context(tc.tile_pool(name="y8", bufs=1))
        wqk_pool = ctx.enter_context(tc.tile_pool(name="wqk", bufs=1))
        v_pool = ctx.enter_context(tc.tile_pool(name="vproj", bufs=1))
        qt_pool = ctx.enter_context(tc.tile_pool(name="qt", bufs=2))
        g8_pool = ctx.enter_context(tc.tile_pool(name="g8", bufs=2))
        gbf_pool = ctx.enter_context(tc.tile_pool(name="gbf", bufs=2))
        eT_pool = ctx.enter_context(tc.tile_pool(name="eT", bufs=2))
        out_pool = ctx.enter_context(tc.tile_pool(name="outsb", bufs=4))
        small = ctx.enter_context(tc.tile_pool(name="small", bufs=8))
        yt_pool = ctx.enter_context(tc.tile_pool(name="yt", bufs=1))
        wkv_pool = ctx.enter_context(tc.tile_pool(name="wkv", bufs=1))
        psum_a = ctx.enter_context(
            tc.tile_pool(name="psum_a", bufs=3, space="PSUM"))
        psum_o = ctx.enter_context(
            tc.tile_pool(name="psum_o", bufs=4, space="PSUM"))
        psum_d = ctx.enter_context(
            tc.tile_pool(name="psum_d", bufs=1, space="PSUM"))

        ones_bf = consts.tile([P, 1], BF16)
        nc.vector.memset(ones_bf, 1.0)
        zbias = consts.tile([P, 1], F32)
        nc.vector.memset(zbias, 0.0)

        y8 = y8_pool.tile([P, KC, nk], FP8)       # [d_p, d_c, m] (d < KD only)
        wqk = wqk_pool.tile([P, DC, DC, P], BF16)  # [d_p, d_c, e_c, e_l]
        v_sb = v_pool.tile([P, MC, f], BF16)      # [m_p, m_c, f]
        yt = yt_pool.tile([P, DC, nk], BF16)      # [d_p, d_c, m]
        wkv = wkv_pool.tile([P, DC, f], BF16)
        warm = consts.tile([P, 512], BF16)
        nc.vector.memset(warm, 0.0)

        # ---- preload DMAs, striped across both queues in consumption
        # order: (qt0 chunk, wqk piece) pairs per d-chunk for the d-major
        # g(0) stream, then y8 + yT tail chunks (S), then the v-phase feed
        # (yT head + wkv). ----
        qt0 = qt_pool.tile([P, DC, nblk], BF16, tag="qt", name="qt0")
        for di in range(DC):
            q_ = nc.sync if di % 2 == 0 else nc.gpsimd
            q_.dma_start(qt0[:, di, :], qT_v[0][:, di, :])
            q_.dma_start(wqk[:, di], Wqk_v[:, di])
        nc.sync.dma_start(y8[:, :KC // 2, :], y8_v[:, :KC // 2, :])
        nc.gpsimd.dma_start(y8[:, KC // 2:, :], y8_v[:, KC // 2:, :])
        for c in range(KC, DC):
            q_ = nc.sync if c % 2 == 0 else nc.gpsimd
            q_.dma_start(yt[:, c, :], yT_v[:, c, :])
        nc.sync.dma_start(yt[:, :KC // 2, :], yT_v[:, :KC // 2, :])
        nc.gpsimd.dma_start(yt[:, KC // 2:KC, :], yT_v[:, KC // 2:KC, :])
        nc.gpsimd.dma_start(wkv, Wkv_v)

        # warm up the tensor engine p-state while the first DMAs land
        for _ in range(12):
            wps = psum_a.tile([P, 512], F32, tag="psa", name="warm")
            nc.tensor.matmul(wps, lhsT=warm[:, 0:P], rhs=warm,
                             start=True, stop=True)

        def g_quant(ei, ps, g8, gbf):
            # quantize psum (=16g): d<KD -> fp8, d>=KD -> bf16 x16
            if ei < KC:
                nc.scalar.activation(g8[:, ei, :], ps,
                                     mybir.ActivationFunctionType.Copy)
            else:
                nc.vector.tensor_scalar_mul(gbf[:, ei - KC, :], ps, GSCALE)

        def g_phase(qt):
            g8 = g8_pool.tile([P, KC, nblk], FP8, tag="g8", name="g8")
            gbf = gbf_pool.tile([P, DC - KC, nblk], BF16, tag="gbf", name="gbf")
            for ei in range(DC):
                ps = psum_a.tile([P, 512], F32, tag="psa", name="psa")
                for di in range(DC):
                    nc.tensor.matmul(
                        ps,
                        lhsT=wqk[:, di, ei, :],
                        rhs=qt[:, di, :],
                        start=(di == 0), stop=(di == DC - 1))
                g_quant(ei, ps, g8, gbf)
            return g8, gbf

        def g_phase_dmajor(qt):
            # Startup variant: all 8 ei-psums live at once (borrowing the
            # idle psum_o/psum_d banks), streaming d-chunk-major so the
            # first matmul needs only the first (qt chunk, wqk piece) pair.
            g8 = g8_pool.tile([P, KC, nblk], FP8, tag="g8", name="g8")
            gbf = gbf_pool.tile([P, DC - KC, nblk], BF16, tag="gbf", name="gbf")
            pss = ([psum_a.tile([P, 512], F32, tag="psa", name="psa")
                    for _ in range(3)] +
                   [psum_o.tile([P, 512], F32, tag="pso", name="pso")
                    for _ in range(4)] +
                   [psum_d.tile([P, 512], F32, tag="pss", name="pssg")])
            for di in range(DC):
                for ei in range(DC):
                    nc.tensor.matmul(
                        pss[ei],
                        lhsT=wqk[:, di, ei, :],
                        rhs=qt[:, di, :],
                        start=(di == 0), stop=(di == DC - 1))
            for ei in range(DC):
                g_quant(ei, pss[ei], g8, gbf)
            return g8, gbf

        def s_phase(g8, gbf):
            # S[m, n] (psum = 256*scores_raw) -> eT = exp(psum/8192), bf16
            eT = eT_pool.tile([P, MC, nblk], BF16, tag="eT", name="eT")
            for mi in range(MC):
                ps = psum_a.tile([P, 512], F32, tag="psa", name="psa")
                for c in range(KC // 2):
                    nc.tensor.matmul(
                        ps,
                        lhsT=y8[:, 2 * c:2 * c + 2, mi * P:(mi + 1) * P],
                        rhs=g8[:, 2 * c:2 * c + 2, :],
                        start=(c == 0), stop=False,
                        perf_mode=mybir.MatmulPerfMode.DoubleRow)
                for c in range(DC - KC):
                    nc.tensor.matmul(
                        ps,
                        lhsT=yt[:, KC + c, mi * P:(mi + 1) * P],
                        rhs=gbf[:, c, :],
                        start=False, stop=(c == DC - KC - 1))
                nc.scalar.activation(
                    eT[:, mi, :], ps,
                    mybir.ActivationFunctionType.Exp,
                    bias=zbias, scale=EXP_SCALE)
            return eT

        # ---- g(0) + S(0) first: they only need 4MB of DMA, so the tensor
        # engine starts ~8us in instead of waiting for the v-phase feed ----
        g8_0, gbf_0 = g_phase_dmajor(qt0)
        eT_0 = s_phase(g8_0, gbf_0)

        # ---- v[m, f] = sum_d yT[d, m] * Wkv[d, f]  (bf16) ----
        for fb in range(FB):
            for mi in range(MC):
                ps = psum_a.tile([P, 512], F32, tag="psa", name="psa")
                for di in range(DC):
                    nc.tensor.matmul(
                        ps,
                        lhsT=yt[:, di, mi * P:(mi + 1) * P],
                        rhs=wkv[:, di, fb * 512:(fb + 1) * 512],
                        start=(di == 0), stop=(di == DC - 1))
                nc.vector.tensor_copy(v_sb[:, mi, fb * 512:(fb + 1) * 512], ps)

        # ---- attention, blocked over queries ----
        for nb in range(NB):
            if nb == 0:
                eT = eT_0
            else:
                qt = qt_pool.tile([P, DC, nblk], BF16, tag="qt", name="qt")
                nc.gpsimd.dma_start(qt, qT_v[nb])
                g8, gbf = g_phase(qt)
                eT = s_phase(g8, gbf)

            # out[n, f] = (eT.T @ v) / (eT.T @ 1).  The denominator matmuls
            # interleave between the two 512-wide f-blocks per m-chunk, where
            # they pipeline in for ~35ns each (standalone they cost ~165ns).
            for ns in range(NSUB):
                pos = [psum_o.tile([P, 512], F32, tag="pso", name="pso")
                       for _ in range(FB)]
                pss = psum_d.tile([P, 1], F32, tag="pss", name="pss")
                for mi in range(MC):
                    lhsT_e = eT[:, mi, ns * P:(ns + 1) * P]
                    for fb in range(FB):
                        nc.tensor.matmul(
                            pos[fb], lhsT=lhsT_e,
                            rhs=v_sb[:, mi, fb * 512:(fb + 1) * 512],
                            start=(mi == 0), stop=(mi == MC - 1))
                    nc.tensor.matmul(
                        pss, lhsT=lhsT_e, rhs=ones_bf,
                        start=(mi == 0), stop=(mi == MC - 1))
                rec = small.tile([P, 1], F32)
                nc.vector.reciprocal(rec, pss)
                ob = out_pool.tile([P, f], F32, tag="ob", name="ob")
                for fb in range(FB):
                    nc.vector.tensor_scalar_mul(
                        ob[:, fb * 512:(fb + 1) * 512], pos[fb], rec)
                    nc.sync.dma_start(
                        out_v[nb * NSUB + ns][:, fb * 512:(fb + 1) * 512],
                        ob[:, fb * 512:(fb + 1) * 512])

    nc.compile()
    return nc


def _sbufize(xT):
    """[d, X] row-major -> SBUF-layout blob [128, (d//128)*X] so the DMA is
    a contiguous per-partition blit."""
    dd, X = xT.shape
    c = dd // P
    return np.ascontiguousarray(
        xT.reshape(c, P, X).transpose(1, 0, 2).reshape(P, c * X))


def make_in_maps(q, y, Wq, Wk, Wv):
    """Host prep: weight products, transposes, dtype casts, fp8 quantize."""
    q = np.asarray(q, dtype=np.float32)
    y = np.asarray(y, dtype=np.float32)
    Wq = np.asarray(Wq, dtype=np.float32)
    Wk = np.asarray(Wk, dtype=np.float32)
    Wv = np.asarray(Wv, dtype=np.float32)

    # Wqk: d-chunk-major SBUF layout [p, di, ei, el]
    Wqk16 = (GSCALE * (Wq @ Wk.T)).astype(BF16NP)      # [d, e]
    Wqk = np.ascontiguousarray(
        Wqk16.reshape(8, P, 8, P).transpose(1, 0, 2, 3).reshape(P, 8 * 1024))
    Wkv = _sbufize((Wk @ Wv).astype(BF16NP))

    in_maps = []
    for b in range(B):
        qT = q[b].T.astype(BF16NP)          # [1024, 2048]
        yT = y[b].T
        # per-block SBUF layout: [NB, 128, DC*nblk]
        qTb = np.ascontiguousarray(
            qT.reshape(8, P, 4, 512).transpose(2, 1, 0, 3).reshape(4, P, 8 * 512))
        in_maps.append({
            "qT": qTb,
            "yT": _sbufize(yT.astype(BF16NP)),
            "y8T": _sbufize((YSCALE * yT[:KD]).astype(E4NP)),
            "Wqk": Wqk, "Wkv": Wkv,
        })
    return in_maps


_CACHE = {}


def kernel(q, y, Wq, Wk, Wv):
    if "nc" not in _CACHE:
        _CACHE["nc"] = build_program()
    nc = _CACHE["nc"]
    in_maps = make_in_maps(q, y, Wq, Wk, Wv)
    res = run_bass_kernel_spmd(nc, in_maps, core_ids=list(range(B)))
    return np.stack([res.results[b]["out"] for b in range(B)], axis=0)


# revision 43
# speedup vs baseline: 1.0155x; 1.0155x over previous
"""Cross-attention kernel for Trainium2, 8 NeuronCores, data-parallel over batch.

Reference math per batch b:
    q_proj = q[b] @ Wq;  k_proj = y[b] @ Wk;  v_proj = k_proj @ Wv
    out = softmax(q_proj @ k_proj.T / 32) @ v_proj

Host-side restructure (kills the k-projection entirely, -14% device FLOPs):
    Wqk = Wq @ Wk.T   ->  scores = (q @ Wqk) @ y.T / 32
    Wkv = Wk @ Wv     ->  v_proj = y @ Wkv

Device per core (one batch per core, feature-major "T" layouts, no
on-device transposes):
    g   = q @ (16*Wqk)            bf16 matmul          [d', n] in PSUM (=16g)
    g8  = fp8e4(psum[0:KD])       ScalarE quantize     (fp8 half)
    gbf = bf16(16*psum[KD:])      ScalarE scale-copy   (bf16 half, =256g)
    v   = y @ Wkv                 bf16 matmul          [m, f] bf16
    S   = y8[:KD].T @ g8  (fp8 DoubleRow)  +  y[KD:].T @ gbf  (bf16)
    eT  = exp(S / 8192)           ScalarE              [m, n] bf16
    out = (eT.T @ v) / (eT.T @ 1) bf16 + ones-matmul denominator

fp8e4 DoubleRow contracts 256/instr at the same per-column rate as bf16
-> 2x throughput per pass (measured on hw).  A full-d single fp8 pass
costs 1.9e-2 rel err (too close to the 2e-2 gate); quantizing KD=768 of
the 1024-deep contraction and doing the rest in bf16 lands at 1.55e-2
with 5/8 of the bf16 score cost.  y is quantized on host (scaled by 16
to dodge the e4m3 subnormal range; the scale folds into exp's 1/8192).

All inputs are shipped in exact SBUF layout ([128, bytes] per-partition
blits, Wqk e-chunk-major) and striped across the sync+gpsimd DMA queues
in consumption order, so the tensor engine starts ~7us into the NEFF and
stays >92% busy.  Warmup matmuls ramp the PE clock during the first DMA.
"""

import numpy as np
import ml_dtypes
from contextlib import ExitStack

import concourse.bass as bass
import concourse.tile as tile
from concourse import bacc, mybir
from concourse.bass_utils import run_bass_kernel_spmd

P = 128
F32 = mybir.dt.float32
BF16 = mybir.dt.bfloat16
FP8 = mybir.dt.float8e4
E4NP = ml_dtypes.float8_e4m3
BF16NP = ml_dtypes.bfloat16

# Problem shapes (hardcoded per contract)
B = 8
NQ = 2048
NK = 2048
D = 1024   # in_q_dim == in_dim == hid_q == out_dim
F = 1024

YSCALE = 16.0   # host folds into y8;   |16*y|  < ~90  (e4m3 max 240)
GSCALE = 16.0   # host folds into Wqk;  |16*g|  < ~40
EXP_SCALE = 1.0 / (YSCALE * GSCALE * 32.0)  # exp((S_psum)/8192)
KD = 768        # d-range [0:KD) of the scores contraction runs fp8-DR


def build_program(nq=NQ, nk=NK, d=D, f=F, nblk=512):
    nc = bacc.Bacc(trn_type="TRN2")

    DC = d // P            # contraction chunks (8)
    KC = KD // P           # fp8 chunks of the scores contraction (4)
    MC = nk // P           # key chunks (16)
    NB = nq // nblk        # query blocks (4)
    NSUB = nblk // P       # 128-row subblocks per query block (4)
    FB = f // 512          # value free blocks (2)

    # Inputs are pre-arranged on host into exact SBUF layout [128, ...] so
    # every input DMA is a contiguous per-partition blit (multi-KB lines).
    NB_ = nq // nblk
    qT = nc.dram_tensor("qT", [NB_, P, DC * nblk], BF16, kind="ExternalInput").ap()
    yT = nc.dram_tensor("yT", [P, DC * nk], BF16, kind="ExternalInput").ap()
    y8T = nc.dram_tensor("y8T", [P, KC * nk], FP8, kind="ExternalInput").ap()
    Wqk = nc.dram_tensor("Wqk", [P, DC * d], BF16, kind="ExternalInput").ap()
    Wkv = nc.dram_tensor("Wkv", [P, DC * f], BF16, kind="ExternalInput").ap()
    out = nc.dram_tensor("out", [nq, f], F32, kind="ExternalOutput").ap()

    qT_v = qT.rearrange("b p (c n) -> b p c n", c=DC)
    yT_v = yT.rearrange("p (c m) -> p c m", c=DC)
    y8_v = y8T.rearrange("p (c m) -> p c m", c=KC)
    # Wqk host layout is e-chunk-major [p, ei, di, el] so the g-phase can
    # start on piece ei=0 after ~256KB of DMA instead of the full 2MB.
    Wqk_v = Wqk.rearrange("p (e c l) -> p e c l", e=DC, c=DC)
    Wkv_v = Wkv.rearrange("p (c f) -> p c f", c=DC)
    out_v = out.rearrange("(b p) f -> b p f", p=P)

    with tile.TileContext(nc) as tc, ExitStack() as ctx:
        consts = ctx.enter_context(tc.tile_pool(name="consts", bufs=1))
        y8_pool = ctx.enter_context(tc.tile_pool(name="y8", bufs=1))
        wqk_pool = ctx.enter_context(tc.tile_pool(name="wqk", bufs=1))
        v_pool = ctx.enter_context(tc.tile_pool(name="vproj", bufs=1))
        qt_pool = ctx.enter_context(tc.tile_pool(name="qt", bufs=2))
        g8_pool = ctx.enter_context(tc.tile_pool(name="g8", bufs=2))
        gbf_pool = ctx.enter_context(tc.tile_pool(name="gbf", bufs=2))
        eT_pool = ctx.enter_context(tc.tile_pool(name="eT", bufs=2))
        out_pool = ctx.enter_context(tc.tile_pool(name="outsb", bufs=4))
        small = ctx.enter_context(tc.tile_pool(name="small", bufs=8))
        yt_pool = ctx.enter_context(tc.tile_pool(name="yt", bufs=1))
        wkv_pool = ctx.enter_context(tc.tile_pool(name="wkv", bufs=1))
        psum_a = ctx.enter_context(
            tc.tile_pool(name="psum_a", bufs=3, space="PSUM"))
        psum_o = ctx.enter_context(
            tc.tile_pool(name="psum_o", bufs=4, space="PSUM"))
        psum_d = ctx.enter_context(
            tc.tile_pool(name="psum_d", bufs=1, space="PSUM"))

        ones_bf = consts.tile([P, 1], BF16)
        nc.vector.memset(ones_bf, 1.0)
        zbias = consts.tile([P, 1], F32)
        nc.vector.memset(zbias, 0.0)

        y8 = y8_pool.tile([P, KC, nk], FP8)       # [d_p, d_c, m] (d < KD only)
        wqk = wqk_pool.tile([P, DC, DC, P], BF16)  # [d_p, e_c, d_c, e_l]
        v_sb = v_pool.tile([P, MC, f], BF16)      # [m_p, m_c, f]
        yt = yt_pool.tile([P, DC, nk], BF16)      # [d_p, d_c, m]
        wkv = wkv_pool.tile([P, DC, f], BF16)
        warm = consts.tile([P, 512], BF16)
        nc.vector.memset(warm, 0.0)

        # ---- preload DMAs, striped across both queues in consumption
        # order: (qt0 chunk, wqk piece) pairs per d-chunk for the d-major
        # g(0) stream, then y8 + yT tail chunks (S), then the v-phase feed
        # (yT head + wkv). ----
        # scalar queue is idle until the out-DMAs (~90us in), so it carries
        # a third of the preload — wqk piece-0 lands on it first.
        qt0 = qt_pool.tile([P, DC, nblk], BF16, tag="qt", name="qt0")
        nc.scalar.dma_start(wqk[:, 0], Wqk_v[:, 0])
        nc.sync.dma_start(qt0[:, :DC // 2, :], qT_v[0][:, :DC // 2, :])
        nc.gpsimd.dma_start(qt0[:, DC // 2:, :], qT_v[0][:, DC // 2:, :])
        Q3 = [nc.sync, nc.gpsimd, nc.scalar]
        for ei in range(1, DC):
            Q3[(ei - 1) % 3].dma_start(wqk[:, ei], Wqk_v[:, ei])
        for h in range(3):
            lo = (KC * h) // 3
            hi = (KC * (h + 1)) // 3
            if hi > lo:
                Q3[h].dma_start(y8[:, lo:hi, :], y8_v[:, lo:hi, :])
        for c in range(KC, DC):
            Q3[c % 3].dma_start(yt[:, c, :], yT_v[:, c, :])
        for c in range(KC):
            Q3[c % 3].dma_start(yt[:, c, :], yT_v[:, c, :])
        nc.sync.dma_start(wkv[:, :DC // 2, :], Wkv_v[:, :DC // 2, :])
        nc.gpsimd.dma_start(wkv[:, DC // 2:, :], Wkv_v[:, DC // 2:, :])

        # warm up the tensor engine p-state while the first DMAs land
        for _ in range(12):
            wps = psum_a.tile([P, 512], F32, tag="psa", name="warm")
            nc.tensor.matmul(wps, lhsT=warm[:, 0:P], rhs=warm,
                             start=True, stop=True)

        def g_quant(ei, ps, g8, gbf):
            # quantize psum (=16g): d<KD -> fp8, d>=KD -> bf16 x16
            if ei < KC:
                nc.scalar.activation(g8[:, ei, :], ps,
                                     mybir.ActivationFunctionType.Copy)
            else:
                nc.vector.tensor_scalar_mul(gbf[:, ei - KC, :], ps, GSCALE)

        def g_phase(qt):
            g8 = g8_pool.tile([P, KC, nblk], FP8, tag="g8", name="g8")
            gbf = gbf_pool.tile([P, DC - KC, nblk], BF16, tag="gbf", name="gbf")
            for ei in range(DC):
                ps = psum_a.tile([P, 512], F32, tag="psa", name="psa")
                for di in range(DC):
                    nc.tensor.matmul(
                        ps,
                        lhsT=wqk[:, ei, di, :],
                        rhs=qt[:, di, :],
                        start=(di == 0), stop=(di == DC - 1))
                g_quant(ei, ps, g8, gbf)
            return g8, gbf

        def s_phase(g8, gbf):
            # S[m, n] (psum = 256*scores_raw) -> eT = exp(psum/8192), bf16
            eT = eT_pool.tile([P, MC, nblk], BF16, tag="eT", name="eT")
            for mi in range(MC):
                ps = psum_a.tile([P, 512], F32, tag="psa", name="psa")
                for c in range(KC // 2):
                    nc.tensor.matmul(
                        ps,
                        lhsT=y8[:, 2 * c:2 * c + 2, mi * P:(mi + 1) * P],
                        rhs=g8[:, 2 * c:2 * c + 2, :],
                        start=(c == 0), stop=False,
                        perf_mode=mybir.MatmulPerfMode.DoubleRow)
                for c in range(DC - KC):
                    nc.tensor.matmul(
                        ps,
                        lhsT=yt[:, KC + c, mi * P:(mi + 1) * P],
                        rhs=gbf[:, c, :],
                        start=False, stop=(c == DC - KC - 1))
                nc.scalar.activation(
                    eT[:, mi, :], ps,
                    mybir.ActivationFunctionType.Exp,
                    bias=zbias, scale=EXP_SCALE)
            return eT

        # ---- g(0) + S(0) first: they only need 4MB of DMA, so the tensor
        # engine starts ~6us in instead of waiting for the v-phase feed ----
        g8_0, gbf_0 = g_phase(qt0)
        eT_0 = s_phase(g8_0, gbf_0)

        # ---- v[m, f] = sum_d yT[d, m] * Wkv[d, f]  (bf16) ----
        for fb in range(FB):
            for mi in range(MC):
                ps = psum_a.tile([P, 512], F32, tag="psa", name="psa")
                for di in range(DC):
                    nc.tensor.matmul(
                        ps,
                        lhsT=yt[:, di, mi * P:(mi + 1) * P],
                        rhs=wkv[:, di, fb * 512:(fb + 1) * 512],
                        start=(di == 0), stop=(di == DC - 1))
                nc.vector.tensor_copy(v_sb[:, mi, fb * 512:(fb + 1) * 512], ps)

        # ---- attention, blocked over queries ----
        for nb in range(NB):
            if nb == 0:
                eT = eT_0
            else:
                qt = qt_pool.tile([P, DC, nblk], BF16, tag="qt", name="qt")
                nc.gpsimd.dma_start(qt, qT_v[nb])
                g8, gbf = g_phase(qt)
                eT = s_phase(g8, gbf)

            # out[n, f] = (eT.T @ v) / (eT.T @ 1).  The denominator matmuls
            # interleave between the two 512-wide f-blocks per m-chunk, where
            # they pipeline in for ~35ns each (standalone they cost ~165ns).
            for ns in range(NSUB):
                pos = [psum_o.tile([P, 512], F32, tag="pso", name="pso")
                       for _ in range(FB)]
                pss = psum_d.tile([P, 1], F32, tag="pss", name="pss")
                for mi in range(MC):
                    lhsT_e = eT[:, mi, ns * P:(ns + 1) * P]
                    for fb in range(FB):
                        nc.tensor.matmul(
                            pos[fb], lhsT=lhsT_e,
                            rhs=v_sb[:, mi, fb * 512:(fb + 1) * 512],
                            start=(mi == 0), stop=(mi == MC - 1))
                    nc.tensor.matmul(
                        pss, lhsT=lhsT_e, rhs=ones_bf,
                        start=(mi == 0), stop=(mi == MC - 1))
                rec = small.tile([P, 1], F32)
                nc.vector.reciprocal(rec, pss)
                ob = out_pool.tile([P, f], F32, tag="ob", name="ob")
                for fb in range(FB):
                    nc.vector.tensor_scalar_mul(
                        ob[:, fb * 512:(fb + 1) * 512], pos[fb], rec)
                    nc.sync.dma_start(
                        out_v[nb * NSUB + ns][:, fb * 512:(fb + 1) * 512],
                        ob[:, fb * 512:(fb + 1) * 512])

    nc.compile()
    return nc


def _sbufize(xT):
    """[d, X] row-major -> SBUF-layout blob [128, (d//128)*X] so the DMA is
    a contiguous per-partition blit."""
    dd, X = xT.shape
    c = dd // P
    return np.ascontiguousarray(
        xT.reshape(c, P, X).transpose(1, 0, 2).reshape(P, c * X))


def make_in_maps(q, y, Wq, Wk, Wv):
    """Host prep: weight products, transposes, dtype casts, fp8 quantize."""
    q = np.asarray(q, dtype=np.float32)
    y = np.asarray(y, dtype=np.float32)
    Wq = np.asarray(Wq, dtype=np.float32)
    Wk = np.asarray(Wk, dtype=np.float32)
    Wv = np.asarray(Wv, dtype=np.float32)

    # Wqk: e-chunk-major SBUF layout [p, ei, di, el]
    Wqk16 = (GSCALE * (Wq @ Wk.T)).astype(BF16NP)      # [d, e]
    Wqk = np.ascontiguousarray(
        Wqk16.reshape(8, P, 8, P).transpose(1, 2, 0, 3).reshape(P, 8 * 1024))
    Wkv = _sbufize((Wk @ Wv).astype(BF16NP))

    in_maps = []
    for b in range(B):
        qT = q[b].T.astype(BF16NP)          # [1024, 2048]
        yT = y[b].T
        # per-block SBUF layout: [NB, 128, DC*nblk]
        qTb = np.ascontiguousarray(
            qT.reshape(8, P, 4, 512).transpose(2, 1, 0, 3).reshape(4, P, 8 * 512))
        in_maps.append({
            "qT": qTb,
            "yT": _sbufize(yT.astype(BF16NP)),
            "y8T": _sbufize((YSCALE * yT[:KD]).astype(E4NP)),
            "Wqk": Wqk, "Wkv": Wkv,
        })
    return in_maps


_CACHE = {}


def kernel(q, y, Wq, Wk, Wv):
    if "nc" not in _CACHE:
        _CACHE["nc"] = build_program()
    nc = _CACHE["nc"]
    in_maps = make_in_maps(q, y, Wq, Wk, Wv)
    res = run_bass_kernel_spmd(nc, in_maps, core_ids=list(range(B)))
    return np.stack([res.results[b]["out"] for b in range(B)], axis=0)
